# revision 1
# baseline (speedup 1.0000x reference)
"""Trainium2 Bass kernel for nn_DeepGATEncoder (3-layer GAT + mean-pool + MLP).

Sharding: the 3072 nodes' attention rows are split 384/core across 8 cores.
Weights are replicated. Each core computes Wh (+ attention-bias column d and a
ones column for the softmax denominator) for its own 384 nodes per head, the
per-head blocks are AllGather'ed, and each core then runs masked softmax
attention for its own rows against all 3072 columns. Between layers only the
locally-owned columns of h^T are needed, so no further gathers. The mean-pool
partials are AllReduce'd and the tiny MLP is computed redundantly on every
core.

Softmax is computed without max-subtraction (safe: e = lrelu(s_i+d_j) stays
< ~15 for glorot-initialized weights at these widths, far below fp32 exp
overflow) via
    exp(lrelu(z)) = max(exp(z), exp(.02 z)),  exp(.02 z) = exp(.02 s)exp(.02 d)
so the inner loop is one ACT exp + three cheap DVE/GPSIMD ops per 128x384
tile, and the softmax denominator falls out of the attention matmul via an
appended ones column.
"""

import os
import numpy as np

import concourse.bass as bass
import concourse.bacc as bacc
import concourse.mybir as mybir
import concourse.tile as tile
from concourse.bass_utils import run_bass_kernel_spmd

# ---- problem constants (hardcoded; kernel.py must be self-contained) ----
N = 3072
F_IN = 300
HID = 300
OUT_ATT = 600
HEADS = 10
N_GRAPHS = 96
MLP_HID = 600
NOUT = 768
ALPHA = 0.02

NCORES = 8
RPC = N // NCORES          # 384 rows (nodes) per core
NJT = RPC // 128           # 3 own-row tiles of 128
NCH = N // 128             # 24 column chunks of 128

F32 = mybir.dt.float32
BF16 = mybir.dt.bfloat16
AF = mybir.ActivationFunctionType

# compute dtype for matmul operands / streamed data
CDT = F32 if os.environ.get("KERNEL_F32") else BF16

TRACE = bool(os.environ.get("KERNEL_TRACE"))
DEBUG_STAGE = os.environ.get("KERNEL_DEBUG", "")

_compiled = {}


class _EarlyExit(Exception):
    pass


def _chunks(total, step=128):
    out = []
    lo = 0
    while lo < total:
        out.append((lo, min(step, total - lo)))
        lo += step
    return out


def _mm(nc, out, lhsT, rhs, **kw):
    """matmul, with float32r (full-rate fp32) when operands are fp32."""
    if lhsT.dtype == F32:
        lhsT = lhsT.bitcast(mybir.dt.float32r)
        rhs = rhs.bitcast(mybir.dt.float32r)
    nc.tensor.matmul(out, lhsT, rhs, **kw)


def build():
    nc = bacc.Bacc("TRN2", target_bir_lowering=False, debug=False,
                   num_devices=NCORES)

    # ---------------- external I/O (per-core data) ----------------
    xT = nc.dram_tensor("xT", [F_IN, RPC], CDT, kind="ExternalInput")
    adjT = nc.dram_tensor("adjT", [N, RPC], CDT, kind="ExternalInput")
    smat = nc.dram_tensor("smat", [RPC, N_GRAPHS], CDT, kind="ExternalInput")
    # per-layer fused weights: [heads, F, 301] with col 300 = W @ a_dst
    R0 = nc.dram_tensor("R0", [HEADS, F_IN, HID + 1], CDT, kind="ExternalInput")
    WA0 = nc.dram_tensor("WA0", [F_IN, HEADS], CDT, kind="ExternalInput")
    R1 = nc.dram_tensor("R1", [HEADS, HEADS * HID, HID + 1], CDT, kind="ExternalInput")
    WA1 = nc.dram_tensor("WA1", [HEADS * HID, HEADS], CDT, kind="ExternalInput")
    Ro = nc.dram_tensor("Ro", [HEADS * HID, OUT_ATT + 1], CDT, kind="ExternalInput")
    WAo = nc.dram_tensor("WAo", [HEADS * HID, 1], CDT, kind="ExternalInput")
    Wm1 = nc.dram_tensor("Wm1", [OUT_ATT, MLP_HID], CDT, kind="ExternalInput")
    bm1 = nc.dram_tensor("bm1", [MLP_HID, 1], F32, kind="ExternalInput")
    Wm2 = nc.dram_tensor("Wm2", [MLP_HID, NOUT], CDT, kind="ExternalInput")
    bm2 = nc.dram_tensor("bm2", [NOUT, 1], F32, kind="ExternalInput")
    eye10f = nc.dram_tensor("eye10f", [HEADS, HEADS * 128], F32, kind="ExternalInput")
    eye10c = nc.dram_tensor("eye10c", [HEADS, HEADS * 128], CDT, kind="ExternalInput")
    outT = nc.dram_tensor("outT", [NOUT, N_GRAPHS], F32, kind="ExternalOutput")
    dbg = None
    if DEBUG_STAGE in ("L0", "L1"):
        dbg = nc.dram_tensor("dbg", [HEADS * HID, RPC], F32, kind="ExternalOutput")
    elif DEBUG_STAGE == "S0":
        dbg = nc.dram_tensor("dbg", [HEADS, RPC], F32, kind="ExternalOutput")
    elif DEBUG_STAGE == "WG0":
        dbg = nc.dram_tensor("dbg", [N, HID + 1], F32, kind="ExternalOutput")
    elif DEBUG_STAGE == "HO":
        dbg = nc.dram_tensor("dbg", [RPC, OUT_ATT], F32, kind="ExternalOutput")
    elif DEBUG_STAGE == "POOL":
        dbg = nc.dram_tensor("dbg", [OUT_ATT, N_GRAPHS], F32, kind="ExternalOutput")

    rg = [list(range(NCORES))]

    with tile.TileContext(nc) as tc:
        with (
            tc.tile_pool(name="persist", bufs=1) as persist,
            tc.tile_pool(name="whbuf", bufs=2) as whbufp,
            tc.tile_pool(name="rstream", bufs=4) as rstream,
            tc.tile_pool(name="ew", bufs=3) as ew,
            tc.tile_pool(name="small", bufs=2) as small,
            tc.tile_pool(name="ps", bufs=1, space="PSUM") as ps,
            tc.tile_pool(name="dram", bufs=1, space="DRAM") as dram,
        ):
            # ---------- persistent SBUF state ----------
            adj_sb = persist.tile([128, NCH * RPC], CDT, name="adj_sb")
            nc.sync.dma_start(adj_sb[:].rearrange("p (c i) -> p c i", i=RPC),
                              adjT[:].rearrange("(c p) i -> p c i", p=128))

            smat_sb = [persist.tile([128, N_GRAPHS], CDT, name=f"smat{i}")
                       for i in range(NJT)]
            for i in range(NJT):
                nc.sync.dma_start(smat_sb[i][:], smat[i * 128:(i + 1) * 128, :])

            eyef_sb = persist.tile([HEADS, HEADS * 128], F32, name="eyef_sb")
            nc.sync.dma_start(eyef_sb[:], eye10f[:])
            eyec_sb = persist.tile([HEADS, HEADS * 128], CDT, name="eyec_sb")
            nc.sync.dma_start(eyec_sb[:], eye10c[:])

            # h^T storage between layers (own columns only), [3000, 384]
            hT = dram.tile([HEADS * HID, RPC], CDT, name="hT")

            def load_ht(src_loader, kch):
                tiles = []
                for ci, (lo, sz) in enumerate(kch):
                    t = rstream.tile([128, RPC], CDT, tag=f"ht_{ci}", bufs=1,
                                     name=f"ht_{ci}", uniquify=True)
                    src_loader(t, lo, sz)
                    tiles.append(t)
                return tiles

            onesf_sb = persist.tile([1, 128], F32, name="onesf_sb")
            nc.vector.memset(onesf_sb[:], 1.0)
            onesc_sb = persist.tile([1, 128], CDT, name="onesc_sb")
            nc.vector.memset(onesc_sb[:], 1.0)
            s_sb = persist.tile([HEADS, RPC], F32, name="s_sb")
            es02_sb = persist.tile([HEADS, RPC], CDT, name="es02_sb")
            sbc_all = persist.tile([128, HEADS * RPC], F32, name="sbc_all")
            e02bc_all = persist.tile([128, HEADS * RPC], CDT, name="e02bc_all")

            # ============ one GAT layer ============
            def gat_layer(lidx, fin, r_dram, wa_dram, ht_loader):
                kch = _chunks(fin)
                nkc = len(kch)
                ht_sb = load_ht(ht_loader, kch)

                # --- s vector for all heads: [heads, own-i] ---
                ps_s = ps.tile([HEADS, RPC], F32, tag="p6")
                for ci, (lo, sz) in enumerate(kch):
                    wa_t = rstream.tile([128, HEADS], CDT, tag="wa", bufs=6)
                    nc.sync.dma_start(wa_t[:sz, :], wa_dram[lo:lo + sz, :])
                    _mm(nc, ps_s[:], wa_t[:sz, :], ht_sb[ci][:sz, :],
                        start=(ci == 0), stop=(ci == nkc - 1))
                nc.scalar.activation(s_sb[:], ps_s[:], AF.Copy)
                nc.scalar.activation(es02_sb[:], ps_s[:], AF.Exp, scale=ALPHA)
                for h in range(HEADS):
                    pb = ps.tile([128, RPC], F32, tag="p6", name=f"pb{h}")
                    nc.tensor.matmul(pb[:], eyef_sb[:, h * 128:(h + 1) * 128],
                                     s_sb[:], start=True, stop=True)
                    nc.scalar.activation(sbc_all[:, h * RPC:(h + 1) * RPC],
                                         pb[:], AF.Copy)
                    pb2 = ps.tile([128, RPC], F32, tag="p6", name=f"pb2{h}")
                    nc.tensor.matmul(pb2[:], eyec_sb[:, h * 128:(h + 1) * 128],
                                     es02_sb[:], start=True, stop=True)
                    nc.scalar.activation(e02bc_all[:, h * RPC:(h + 1) * RPC],
                                         pb2[:], AF.Copy)

                wg_list = []
                # --- per-head: local Wh -> DRAM -> AllGather ---
                for h in range(HEADS):
                    wl = dram.tile([RPC, HID + 2], CDT, name=f"wl{lidx}_{h}")
                    wg = dram.tile([N, HID + 2], CDT, name=f"wg{lidx}_{h}", addr_space="Shared")
                    wg_list.append(wg)
                    wl_sb = small.tile([128, NJT * (HID + 2)], CDT, tag="wl_sb")
                    psw_t = [ps.tile([128, HID + 1], F32, tag=f"p{jt}", name=f"psw{jt}")
                             for jt in range(NJT)]
                    for ci, (lo, sz) in enumerate(kch):
                        r_t = rstream.tile([128, HID + 1], CDT, tag="r", bufs=8)
                        nc.sync.dma_start(r_t[:sz, :], r_dram[h, lo:lo + sz, :])
                        for jt in range(NJT):
                            _mm(nc, psw_t[jt][:],
                                ht_sb[ci][:sz, jt * 128:(jt + 1) * 128],
                                r_t[:sz, :],
                                start=(ci == 0), stop=(ci == nkc - 1))
                    for jt in range(NJT):
                        base = jt * (HID + 2)
                        nc.vector.memset(wl_sb[:, base:base + 1], 1.0)
                        nc.scalar.activation(wl_sb[:, base + 1:base + HID + 2],
                                             psw_t[jt][:], AF.Copy)
                        nc.scalar.dma_start(wl[jt * 128:(jt + 1) * 128, :],
                                            wl_sb[:, base:base + HID + 2])
                    nc.gpsimd.collective_compute(
                        "AllGather", mybir.AluOpType.bypass, replica_groups=rg,
                        ins=[wl.opt()], outs=[wg.opt()])

                # --- per-head attention over gathered Wh ---
                # Software-pipelined: head h's divide/ELU/store is emitted
                # after head h+1's matmul phase so PE never starves at head
                # boundaries.
                W2 = HID + 2
                osp = [(0, 128), (128, 128), (256, 45)]

                def att_front(h):
                    wg = wg_list[h]
                    whb = whbufp.tile([128, NCH * W2], CDT, tag="whb",
                                      name=f"whb{h}")
                    nc.sync.dma_start(
                        whb[:].rearrange("p (c w) -> p c w", w=W2),
                        wg[:].rearrange("(c p) w -> p c w", p=128))
                    ed02 = small.tile([128, NCH], F32, tag="ed02",
                                      name=f"ed02_{h}")
                    dcols = whb[:].rearrange("p (c w) -> p c w", w=W2)[:, :, HID + 1]
                    nc.scalar.activation(ed02[:], dcols, AF.Exp, scale=ALPHA)
                    sbc = sbc_all[:, h * RPC:(h + 1) * RPC]
                    e02bc = e02bc_all[:, h * RPC:(h + 1) * RPC]

                    ps_att = [ps.tile([128, RPC], F32, tag=f"p{3 + om}",
                                      name=f"psatt{om}")
                              for om in range(len(osp))]
                    GRP = 3
                    for c0 in range(0, NCH, GRP):
                        a_t = ew.tile([128, GRP * RPC], CDT, tag="a", bufs=2)
                        b_t = ew.tile([128, GRP * RPC], CDT, tag="b", bufs=2)
                        for k in range(GRP):
                            c = c0 + k
                            nc.scalar.activation(
                                a_t[:, k * RPC:(k + 1) * RPC], sbc, AF.Exp,
                                bias=whb[:, c * W2 + HID + 1:c * W2 + HID + 2])
                            nc.vector.tensor_scalar_mul(
                                b_t[:, k * RPC:(k + 1) * RPC], e02bc,
                                ed02[:, c:c + 1])
                        nc.vector.tensor_max(b_t[:], a_t[:], b_t[:])
                        p_t = ew.tile([128, GRP * RPC], CDT, tag="p", bufs=5)
                        nc.vector.tensor_mul(
                            p_t[:], b_t[:],
                            adj_sb[:, c0 * RPC:(c0 + GRP) * RPC])
                        for k in range(GRP):
                            c = c0 + k
                            for om, (lo, sz) in enumerate(osp):
                                _mm(nc, ps_att[om][:sz, :],
                                    whb[:, c * W2 + lo:c * W2 + lo + sz],
                                    p_t[:, k * RPC:(k + 1) * RPC],
                                    start=(c == 0), stop=(c == NCH - 1))
                    # free PSUM fast: copy raw sums to SBUF
                    att32 = [ew.tile([128, RPC], F32, tag=f"att32_{om}",
                                     name=f"att32_{om}", bufs=2)
                             for om in range(len(osp))]
                    with tc.high_priority():
                        for om, (lo, sz) in enumerate(osp):
                            nc.scalar.activation(att32[om][:sz, :],
                                                 ps_att[om][:sz, :], AF.Copy)
                    return att32

                def att_back(h, att32):
                    scr = small.tile([1, RPC], F32, tag="scr")
                    rden = small.tile([1, RPC], F32, tag="rden")
                    nc.vector.reciprocal_approx_accurate(
                        rden[:], att32[0][0:1, :], scr[:])
                    rbc = ps.tile([128, RPC], F32, tag="p7", name="rbc")
                    nc.tensor.matmul(rbc[:], onesf_sb[:], rden[:],
                                     start=True, stop=True)
                    # om0 rows: 0=denominator junk, 1..127 = Wh outs 0..126
                    store = [(0, 0, 128, 1), (1, 127, 128, 0), (2, 255, 45, 0)]
                    for om, hlo, rows, roff in store:
                        y_t = ew.tile([128, RPC], CDT, tag="y")
                        nc.vector.tensor_mul(y_t[:rows, :], att32[om][:rows, :],
                                             rbc[:rows, :])
                        q_t = ew.tile([128, RPC], CDT, tag="q")
                        nc.scalar.activation(q_t[:rows, :], y_t[:rows, :], AF.Exp)
                        nc.vector.tensor_scalar(q_t[:rows, :], q_t[:rows, :],
                                                -1.0, 0.0, mybir.AluOpType.add,
                                                mybir.AluOpType.min)
                        nc.vector.tensor_max(y_t[:rows, :], y_t[:rows, :],
                                             q_t[:rows, :])
                        nrows = rows - roff
                        nc.scalar.dma_start(
                            hT[h * HID + hlo:h * HID + hlo + nrows, :],
                            y_t[roff:rows, :])

                pend = None
                for h in range(HEADS):
                    a32 = att_front(h)
                    if pend is not None:
                        att_back(*pend)
                    pend = (h, a32)
                att_back(*pend)
                return wg_list[0]

            def _tail():
                # ---------------- output attention layer ----------------
                FO = HEADS * HID
                kch = _chunks(FO)
                nkc = len(kch)
                ht_sb = load_ht(
                    lambda t, lo, sz: nc.sync.dma_start(t[:sz, :], hT[lo:lo + sz, :]),
                    kch)
                # s vector
                ps_so = ps.tile([1, RPC], F32, tag="p6")
                for ci, (lo, sz) in enumerate(kch):
                    wa_t = rstream.tile([128, 1], CDT, tag="wa", bufs=6)
                    nc.sync.dma_start(wa_t[:sz, :], WAo[lo:lo + sz, :])
                    _mm(nc, ps_so[:], wa_t[:sz, :], ht_sb[ci][:sz, :],
                        start=(ci == 0), stop=(ci == nkc - 1))
                so_sb = persist.tile([1, RPC], F32, name="so_sb")
                nc.scalar.activation(so_sb[:], ps_so[:], AF.Copy)
                eso02 = persist.tile([1, RPC], CDT, name="eso02")
                nc.scalar.activation(eso02[:], ps_so[:], AF.Exp, scale=ALPHA)

                # local Wh_out [RPC, 602] -> 3 AllGathers (one per own j-tile)
                WO2 = OUT_ATT + 2
                wlo_p = [dram.tile([128, WO2], CDT, name=f"wlo{jt}")
                         for jt in range(NJT)]
                wgo_p = [dram.tile([N // NJT, WO2], CDT, name=f"wgo{jt}",
                                   addr_space="Shared") for jt in range(NJT)]
                nsp = _chunks(OUT_ATT + 1, 512)  # [(0,512),(512,89)]
                wlo_sb = small.tile([128, NJT * WO2], CDT, tag="wlo_sb")
                for jt in range(NJT):
                    pswo = [ps.tile([128, sz], F32, tag=f"p{si}", name=f"pswo{si}")
                            for si, (lo, sz) in enumerate(nsp)]
                    for ci, (lo, sz) in enumerate(kch):
                        r_t = rstream.tile([128, OUT_ATT + 1], CDT, tag="ro")
                        nc.sync.dma_start(r_t[:sz, :], Ro[lo:lo + sz, :])
                        for si, (slo, ssz) in enumerate(nsp):
                            _mm(nc, pswo[si][:],
                                ht_sb[ci][:sz, jt * 128:(jt + 1) * 128],
                                r_t[:sz, slo:slo + ssz],
                                start=(ci == 0), stop=(ci == nkc - 1))
                    base = jt * WO2
                    for si, (slo, ssz) in enumerate(nsp):
                        nc.scalar.activation(wlo_sb[:, base + slo:base + slo + ssz],
                                             pswo[si][:], AF.Copy)
                    nc.vector.memset(wlo_sb[:, base + WO2 - 1:base + WO2], 1.0)
                    nc.scalar.dma_start(wlo_p[jt][:], wlo_sb[:, base:base + WO2])
                    nc.gpsimd.collective_compute(
                        "AllGather", mybir.AluOpType.bypass, replica_groups=rg,
                        ins=[wlo_p[jt].opt()], outs=[wgo_p[jt].opt()])

                # attention (single head), row-major output for pooling
                pbo = ps.tile([128, RPC], F32, tag="p6", name="pbo")
                nc.tensor.matmul(pbo[:], onesf_sb[:], so_sb[:],
                                 start=True, stop=True)
                sbco = ew.tile([128, RPC], F32, tag="sbc")
                nc.scalar.activation(sbco[:], pbo[:], AF.Copy)
                pbo2 = ps.tile([128, RPC], F32, tag="p6", name="pbo2")
                nc.tensor.matmul(pbo2[:], onesc_sb[:], eso02[:],
                                 start=True, stop=True)
                e02bco = ew.tile([128, RPC], CDT, tag="e02bc")
                nc.scalar.activation(e02bco[:], pbo2[:], AF.Copy)

                onsp = _chunks(OUT_ATT + 2, 512)  # [(0,512),(512,90)]
                ps_o = [[ps.tile([128, sz], F32, tag=f"p{it * 2 + si}", name=f"pso{it}_{si}")
                         for si, (lo, sz) in enumerate(onsp)] for it in range(NJT)]
                GRP = 3
                for c0 in range(0, NCH, GRP):
                    whcs = []
                    a_t = ew.tile([128, GRP * RPC], CDT, tag="a", bufs=2)
                    b_t = ew.tile([128, GRP * RPC], CDT, tag="b", bufs=2)
                    for k in range(GRP):
                        c = c0 + k
                        whc = rstream.tile([128, WO2], CDT, tag=f"whc{k}",
                                           name=f"whc{c}", bufs=2)
                        nc.sync.dma_start(
                            whc[:],
                            wgo_p[c % NJT][(c // NJT) * 128:(c // NJT + 1) * 128, :])
                        whcs.append(whc)
                        edo02c = small.tile([128, 1], F32, tag="ed02c", bufs=3)
                        nc.scalar.activation(edo02c[:], whc[:, OUT_ATT:OUT_ATT + 1],
                                             AF.Exp, scale=ALPHA)
                        nc.scalar.activation(
                            a_t[:, k * RPC:(k + 1) * RPC], sbco[:], AF.Exp,
                            bias=whc[:, OUT_ATT:OUT_ATT + 1])
                        nc.vector.tensor_scalar_mul(
                            b_t[:, k * RPC:(k + 1) * RPC], e02bco[:], edo02c[:])
                    nc.vector.tensor_max(b_t[:], a_t[:], b_t[:])
                    p_t = ew.tile([128, GRP * RPC], CDT, tag="p", bufs=5)
                    nc.vector.tensor_mul(p_t[:], b_t[:],
                                         adj_sb[:, c0 * RPC:(c0 + GRP) * RPC])
                    for k in range(GRP):
                        c = c0 + k
                        for it in range(NJT):
                            for si, (slo, ssz) in enumerate(onsp):
                                _mm(nc, ps_o[it][si][:],
                                    p_t[:, k * RPC + it * 128:k * RPC + (it + 1) * 128],
                                    whcs[k][:, slo:slo + ssz],
                                    start=(c == 0), stop=(c == NCH - 1))

                # divide + ELU + pooled^T partial via matmul with smat
                gsp = _chunks(OUT_ATT)  # 600 -> 128x4 + 88
                att_tiles = []
                for it in range(NJT):
                    denc = small.tile([128, 1], F32, tag="denc")
                    scrc = small.tile([128, 1], F32, tag="scrc")
                    rdenc = small.tile([128, 1], F32, tag="rdenc")
                    nc.vector.tensor_copy(denc[:], ps_o[it][-1][:, 89:90])
                    nc.vector.reciprocal_approx_accurate(rdenc[:], denc[:], scrc[:])
                    att_sb = ew.tile([128, OUT_ATT], CDT, tag=f"atts{it}",
                                     name=f"atts{it}", bufs=1)
                    for si, (slo, ssz) in enumerate(onsp):
                        cols = min(ssz, OUT_ATT - slo)
                        y = att_sb[:, slo:slo + cols]
                        nc.vector.tensor_scalar_mul(y, ps_o[it][si][:, :cols], rdenc[:])
                        q_t = ew.tile([128, OUT_ATT], CDT, tag="qo")
                        nc.scalar.activation(q_t[:, :cols], y, AF.Exp)
                        nc.vector.tensor_scalar(q_t[:, :cols], q_t[:, :cols],
                                                -1.0, 0.0, mybir.AluOpType.add,
                                                mybir.AluOpType.min)
                        nc.vector.tensor_max(y, y, q_t[:, :cols])
                    if DEBUG_STAGE == "HO":
                        a32 = ew.tile([128, OUT_ATT], F32, tag="a32dbg")
                        nc.vector.tensor_copy(a32[:], att_sb[:])
                        nc.sync.dma_start(dbg[it * 128:(it + 1) * 128, :], a32[:])
                    att_tiles.append(att_sb)

                # pooled^T [600, 96] -> AllReduce
                pool_l = dram.tile([OUT_ATT, N_GRAPHS], F32, name="pool_l")
                pool_g = dram.tile([OUT_ATT, N_GRAPHS], F32, name="pool_g", addr_space="Shared")
                for g, (glo, gsz) in enumerate(gsp):
                    psp = ps.tile([128, N_GRAPHS], F32, tag="p7", name=f"psp{g}")
                    for it in range(NJT):
                        _mm(nc, psp[:gsz, :], att_tiles[it][:, glo:glo + gsz],
                            smat_sb[it][:], start=(it == 0), stop=(it == NJT - 1))
                    pl_sb = small.tile([128, N_GRAPHS], F32, tag="pl_sb")
                    nc.scalar.activation(pl_sb[:gsz, :], psp[:gsz, :], AF.Copy)
                    nc.sync.dma_start(pool_l[glo:glo + gsz, :], pl_sb[:gsz, :])
                nc.gpsimd.collective_compute(
                    "AllReduce", mybir.AluOpType.add, replica_groups=rg,
                    ins=[pool_l.opt()], outs=[pool_g.opt()])
                if DEBUG_STAGE == "POOL":
                    nc.sync.dma_start(dbg[:], pool_g[:])

                # ---------------- MLP (replicated on every core) ----------------
                pg_sb = []
                for g, (glo, gsz) in enumerate(gsp):
                    t32 = small.tile([128, N_GRAPHS], F32, tag="pg32")
                    nc.sync.dma_start(t32[:gsz, :], pool_g[glo:glo + gsz, :])
                    t = persist.tile([128, N_GRAPHS], CDT, name=f"pg{g}")
                    nc.vector.tensor_copy(t[:gsz, :], t32[:gsz, :])
                    pg_sb.append(t)
                wm1_sb = []
                for g, (glo, gsz) in enumerate(gsp):
                    w = persist.tile([128, MLP_HID], CDT, name=f"wm1_{g}")
                    nc.sync.dma_start(w[:gsz, :], Wm1[glo:glo + gsz, :])
                    wm1_sb.append(w)
                msp = _chunks(MLP_HID)
                bm1_sb = []
                for m, (mlo, msz) in enumerate(msp):
                    b = persist.tile([128, 1], F32, name=f"bm1_{m}")
                    nc.sync.dma_start(b[:msz, :], bm1[mlo:mlo + msz, :])
                    bm1_sb.append(b)
                h1_sb = []
                for m, (mlo, msz) in enumerate(msp):
                    psm = ps.tile([128, N_GRAPHS], F32, tag="p7")
                    for g, (glo, gsz) in enumerate(gsp):
                        _mm(nc, psm[:msz, :], wm1_sb[g][:gsz, mlo:mlo + msz],
                            pg_sb[g][:gsz, :], start=(g == 0), stop=(g == len(gsp) - 1))
                    t = persist.tile([128, N_GRAPHS], CDT, name=f"h1_{m}")
                    nc.scalar.activation(t[:msz, :], psm[:msz, :], AF.Relu,
                                         bias=bm1_sb[m][:msz, :])
                    h1_sb.append(t)
                wm2_sb = []
                for m, (mlo, msz) in enumerate(msp):
                    w = persist.tile([128, NOUT], CDT, name=f"wm2_{m}")
                    nc.sync.dma_start(w[:msz, :], Wm2[mlo:mlo + msz, :])
                    wm2_sb.append(w)
                for o, (olo, osz) in enumerate(_chunks(NOUT)):
                    b2 = small.tile([128, 1], F32, tag="bm2")
                    nc.sync.dma_start(b2[:osz, :], bm2[olo:olo + osz, :])
                    psm = ps.tile([128, N_GRAPHS], F32, tag="p7")
                    for m, (mlo, msz) in enumerate(msp):
                        _mm(nc, psm[:osz, :], wm2_sb[m][:msz, olo:olo + osz],
                            h1_sb[m][:msz, :], start=(m == 0), stop=(m == len(msp) - 1))
                    ot = small.tile([128, N_GRAPHS], F32, tag="ot")
                    nc.vector.tensor_scalar_add(ot[:osz, :], psm[:osz, :], b2[:osz, :])
                    nc.sync.dma_start(outT[olo:olo + osz, :], ot[:osz, :])


            # ---------------- layers 0, 1 ----------------
            def dump_ht():
                for ci, (lo, sz) in enumerate(_chunks(HEADS * HID)):
                    tdb = rstream.tile([128, RPC], CDT, tag="tdb", name=f"tdb{ci}")
                    nc.sync.dma_start(tdb[:sz, :], hT[lo:lo + sz, :])
                    t32 = rstream.tile([128, RPC], F32, tag="tdb32", name=f"t32{ci}")
                    nc.vector.tensor_copy(t32[:sz, :], tdb[:sz, :])
                    nc.sync.dma_start(dbg[lo:lo + sz, :], t32[:sz, :])

            wg00 = gat_layer(0, F_IN, R0, WA0,
                      lambda t, lo, sz: nc.sync.dma_start(t[:sz, :], xT[lo:lo + sz, :]))
            stop = False
            if DEBUG_STAGE == "L0":
                dump_ht()
                stop = True
            elif DEBUG_STAGE == "S0":
                s32 = persist.tile([HEADS, RPC], F32, name="s32dbg")
                nc.vector.tensor_copy(s32[:], s_sb[:])
                nc.sync.dma_start(dbg[:], s32[:])
                stop = True
            elif DEBUG_STAGE == "WG0":
                for ci in range(NCH):
                    tdb = rstream.tile([128, HID + 1], CDT, tag="tdb", name=f"tdb{ci}")
                    nc.sync.dma_start(tdb[:], wg00[ci * 128:(ci + 1) * 128, :])
                    t32 = rstream.tile([128, HID + 1], F32, tag="tdb32", name=f"t32{ci}")
                    nc.vector.tensor_copy(t32[:], tdb[:])
                    nc.sync.dma_start(dbg[ci * 128:(ci + 1) * 128, :], t32[:])
                stop = True
            if not stop:
                gat_layer(1, HEADS * HID, R1, WA1,
                          lambda t, lo, sz: nc.sync.dma_start(t[:sz, :], hT[lo:lo + sz, :]))
                if DEBUG_STAGE == "L1":
                    dump_ht()
                    stop = True
            if not stop:
                _tail()

    nc.compile()
    return nc


# ======================= host side =======================

def _np_cdt(a):
    if CDT == F32:
        return np.ascontiguousarray(a, dtype=np.float32)
    import ml_dtypes
    return np.ascontiguousarray(np.asarray(a, np.float32).astype(ml_dtypes.bfloat16))


def _prep_inputs(x, edge_index, batch, W0, a0_src, a0_dst, W1, a1_src, a1_dst,
                 W_out, ao_src, ao_dst, Wm1, bm1, Wm2, bm2):
    x = np.asarray(x, np.float32)
    ei = np.asarray(edge_index)
    batch = np.asarray(batch).astype(np.int64)
    adj = np.zeros((N, N), np.float32)
    adj[ei[0], ei[1]] = 1.0

    cnt = np.bincount(batch, minlength=N_GRAPHS).astype(np.float32)
    cnt = np.maximum(cnt, 1.0)
    smat_full = np.zeros((N, N_GRAPHS), np.float32)
    smat_full[np.arange(N), batch] = 1.0 / cnt[batch]

    W0 = np.asarray(W0, np.float32)
    W1 = np.asarray(W1, np.float32)
    W_out = np.asarray(W_out, np.float32)

    def fuse(W, a_dst):  # [H, F, O], [H, O] -> [H, F, O+1]
        wad = np.einsum('hfo,ho->hf', W, np.asarray(a_dst, np.float32))
        return np.concatenate([W, wad[:, :, None]], axis=2)

    R0p = fuse(W0, a0_dst)
    WA0p = np.einsum('hfo,ho->fh', W0, np.asarray(a0_src, np.float32))
    R1p = fuse(W1, a1_dst)
    WA1p = np.einsum('hfo,ho->fh', W1, np.asarray(a1_src, np.float32))
    Rop = np.concatenate(
        [W_out, (W_out @ np.asarray(ao_dst, np.float32))[:, None]], axis=1)
    WAop = (W_out @ np.asarray(ao_src, np.float32))[:, None]

    shared = dict(
        R0=_np_cdt(R0p), WA0=_np_cdt(WA0p), R1=_np_cdt(R1p), WA1=_np_cdt(WA1p),
        Ro=_np_cdt(Rop), WAo=_np_cdt(WAop),
        Wm1=_np_cdt(Wm1), bm1=np.ascontiguousarray(np.asarray(bm1, np.float32)[:, None]),
        Wm2=_np_cdt(Wm2), bm2=np.ascontiguousarray(np.asarray(bm2, np.float32)[:, None]),
    )
    eye = np.kron(np.eye(HEADS, dtype=np.float32), np.ones((1, 128), np.float32))
    shared["eye10f"] = np.ascontiguousarray(eye)
    shared["eye10c"] = _np_cdt(eye)
    xT_full = x.T
    in_maps = []
    for c in range(NCORES):
        rows = slice(c * RPC, (c + 1) * RPC)
        m = dict(shared)
        m["xT"] = _np_cdt(xT_full[:, rows])
        m["adjT"] = _np_cdt(adj[rows, :].T)
        m["smat"] = _np_cdt(smat_full[rows, :])
        in_maps.append(m)
    return in_maps


_last_results = None


def kernel(**inputs):
    global _last_results
    if "k" not in _compiled:
        _compiled["k"] = build()
    nc = _compiled["k"]
    in_maps = _prep_inputs(**inputs)
    kw = {}
    if TRACE:
        import tracehook
        tracehook.install()
        kw = dict(trace=True)
        td = os.environ.get("KERNEL_TRACEDIR")
        if td:
            kw["tmpdir"] = td
    res = run_bass_kernel_spmd(nc, in_maps, core_ids=list(range(NCORES)), **kw)
    _last_results = res
    return np.ascontiguousarray(res.results[0]["outT"].T)



# revision 7
# speedup vs baseline: 1.0857x; 1.0857x over previous
"""Trainium2 Bass kernel for nn_DeepGATEncoder (3-layer GAT + mean-pool + MLP).

Sharding: the 3072 nodes' attention rows are split 384/core across 8 cores.
Weights are replicated. Each core computes Wh (+ attention-bias column d and a
ones column for the softmax denominator) for its own 384 nodes per head, the
per-head blocks are AllGather'ed, and each core then runs masked softmax
attention for its own rows against all 3072 columns. Between layers only the
locally-owned columns of h^T are needed, so no further gathers. The mean-pool
partials are AllReduce'd and the tiny MLP is computed redundantly on every
core.

Softmax is computed without max-subtraction (safe: e = lrelu(s_i+d_j) stays
< ~15 for glorot-initialized weights at these widths, far below fp32 exp
overflow) via
    exp(lrelu(z)) = max(exp(z), exp(.02 z)),  exp(.02 z) = exp(.02 s)exp(.02 d)
so the inner loop is one ACT exp + three cheap DVE/GPSIMD ops per 128x384
tile, and the softmax denominator falls out of the attention matmul via an
appended ones column.
"""

import os
import numpy as np

import concourse.bass as bass
import concourse.bacc as bacc
import concourse.mybir as mybir
import concourse.tile as tile
from concourse.bass_utils import run_bass_kernel_spmd

# ---- problem constants (hardcoded; kernel.py must be self-contained) ----
N = 3072
F_IN = 300
HID = 300
OUT_ATT = 600
HEADS = 10
N_GRAPHS = 96
MLP_HID = 600
NOUT = 768
ALPHA = 0.02

NCORES = 8
RPC = N // NCORES          # 384 rows (nodes) per core
NJT = RPC // 128           # 3 own-row tiles of 128
NCH = N // 128             # 24 column chunks of 128

F32 = mybir.dt.float32
BF16 = mybir.dt.bfloat16
AF = mybir.ActivationFunctionType

# compute dtype for matmul operands / streamed data
CDT = F32 if os.environ.get("KERNEL_F32") else BF16

TRACE = bool(os.environ.get("KERNEL_TRACE"))
DEBUG_STAGE = os.environ.get("KERNEL_DEBUG", "")

_compiled = {}


class _EarlyExit(Exception):
    pass


def _chunks(total, step=128):
    out = []
    lo = 0
    while lo < total:
        out.append((lo, min(step, total - lo)))
        lo += step
    return out


def _mm(nc, out, lhsT, rhs, **kw):
    """matmul, with float32r (full-rate fp32) when operands are fp32."""
    if lhsT.dtype == F32:
        lhsT = lhsT.bitcast(mybir.dt.float32r)
        rhs = rhs.bitcast(mybir.dt.float32r)
    nc.tensor.matmul(out, lhsT, rhs, **kw)


def build():
    nc = bacc.Bacc("TRN2", target_bir_lowering=False, debug=False,
                   num_devices=NCORES)

    # ---------------- external I/O (per-core data) ----------------
    xT = nc.dram_tensor("xT", [F_IN, RPC], CDT, kind="ExternalInput")
    adjT = nc.dram_tensor("adjT", [N, RPC], CDT, kind="ExternalInput")
    smat = nc.dram_tensor("smat", [RPC, N_GRAPHS], CDT, kind="ExternalInput")
    # per-layer fused weights: [heads, F, 301] with col 300 = W @ a_dst
    R0 = nc.dram_tensor("R0", [HEADS, F_IN, HID + 1], CDT, kind="ExternalInput")
    WA0 = nc.dram_tensor("WA0", [F_IN, HEADS], CDT, kind="ExternalInput")
    R1 = nc.dram_tensor("R1", [HEADS, HEADS * HID, HID + 1], CDT, kind="ExternalInput")
    WA1 = nc.dram_tensor("WA1", [HEADS * HID, HEADS], CDT, kind="ExternalInput")
    Ro = nc.dram_tensor("Ro", [HEADS * HID, OUT_ATT + 1], CDT, kind="ExternalInput")
    WAo = nc.dram_tensor("WAo", [HEADS * HID, 1], CDT, kind="ExternalInput")
    Wm1 = nc.dram_tensor("Wm1", [OUT_ATT, MLP_HID], CDT, kind="ExternalInput")
    bm1 = nc.dram_tensor("bm1", [MLP_HID, 1], F32, kind="ExternalInput")
    Wm2 = nc.dram_tensor("Wm2", [MLP_HID, NOUT], CDT, kind="ExternalInput")
    bm2 = nc.dram_tensor("bm2", [NOUT, 1], F32, kind="ExternalInput")
    eye10f = nc.dram_tensor("eye10f", [HEADS, HEADS * 128], F32, kind="ExternalInput")
    eye10c = nc.dram_tensor("eye10c", [HEADS, HEADS * 128], CDT, kind="ExternalInput")
    outT = nc.dram_tensor("outT", [NOUT, N_GRAPHS], F32, kind="ExternalOutput")
    dbg = None
    if DEBUG_STAGE in ("L0", "L1"):
        dbg = nc.dram_tensor("dbg", [HEADS * HID, RPC], F32, kind="ExternalOutput")
    elif DEBUG_STAGE == "S0":
        dbg = nc.dram_tensor("dbg", [HEADS, RPC], F32, kind="ExternalOutput")
    elif DEBUG_STAGE == "WG0":
        dbg = nc.dram_tensor("dbg", [N, HID + 1], F32, kind="ExternalOutput")
    elif DEBUG_STAGE == "HO":
        dbg = nc.dram_tensor("dbg", [RPC, OUT_ATT], F32, kind="ExternalOutput")
    elif DEBUG_STAGE == "POOL":
        dbg = nc.dram_tensor("dbg", [OUT_ATT, N_GRAPHS], F32, kind="ExternalOutput")

    rg = [list(range(NCORES))]

    with tile.TileContext(nc) as tc:
        with (
            tc.tile_pool(name="persist", bufs=1) as persist,
            tc.tile_pool(name="whbuf", bufs=2) as whbufp,
            tc.tile_pool(name="rstream", bufs=4) as rstream,
            tc.tile_pool(name="ew", bufs=3) as ew,
            tc.tile_pool(name="small", bufs=2) as small,
            tc.tile_pool(name="ps", bufs=1, space="PSUM") as ps,
            tc.tile_pool(name="dram", bufs=1, space="DRAM") as dram,
        ):
            # ---------- persistent SBUF state ----------
            adj_sb = persist.tile([128, NCH * RPC], CDT, name="adj_sb")
            nc.sync.dma_start(adj_sb[:].rearrange("p (c i) -> p c i", i=RPC),
                              adjT[:].rearrange("(c p) i -> p c i", p=128))

            smat_sb = [persist.tile([128, N_GRAPHS], CDT, name=f"smat{i}")
                       for i in range(NJT)]
            for i in range(NJT):
                nc.sync.dma_start(smat_sb[i][:], smat[i * 128:(i + 1) * 128, :])

            eyef_sb = persist.tile([HEADS, HEADS * 128], F32, name="eyef_sb")
            nc.sync.dma_start(eyef_sb[:], eye10f[:])
            eyec_sb = persist.tile([HEADS, HEADS * 128], CDT, name="eyec_sb")
            nc.sync.dma_start(eyec_sb[:], eye10c[:])

            # h^T storage between layers (own columns only), [3000, 384],
            # kept resident in SBUF as 128-partition chunks laid side by side:
            # row r of h^T lives at [r % 128, (r // 128) * RPC + i].
            NHC = len(_chunks(HEADS * HID))   # 24 chunks covering 3000 rows
            hT_sb = persist.tile([128, NHC * RPC], CDT, name="hT_sb")

            def ht_view(ci, sz=128):
                return hT_sb[:sz, ci * RPC:ci * RPC + RPC]

            def store_hT(y_t, roff, rows, base):
                # copy y_t[roff:rows, :] into h^T rows [base, base+rows-roff)
                # via SBUF->SBUF DMA (partition shift), splitting at 128-row
                # chunk boundaries.
                n = rows - roff
                done = 0
                while done < n:
                    r = base + done
                    ci, po = divmod(r, 128)
                    take = min(n - done, 128 - po)
                    nc.scalar.dma_start(
                        hT_sb[po:po + take, ci * RPC:(ci + 1) * RPC],
                        y_t[roff + done:roff + done + take, :])
                    done += take

            def load_ht(src_loader, kch):
                tiles = []
                for ci, (lo, sz) in enumerate(kch):
                    t = rstream.tile([128, RPC], CDT, tag=f"ht_{ci}", bufs=1,
                                     name=f"ht_{ci}", uniquify=True)
                    src_loader(t, lo, sz)
                    tiles.append(t)
                return tiles

            onesf_sb = persist.tile([1, 128], F32, name="onesf_sb")
            nc.vector.memset(onesf_sb[:], 1.0)
            onesc_sb = persist.tile([1, 128], CDT, name="onesc_sb")
            nc.vector.memset(onesc_sb[:], 1.0)
            s_sb = persist.tile([HEADS, RPC], F32, name="s_sb")
            es02_sb = persist.tile([HEADS, RPC], CDT, name="es02_sb")
            sbc_all = persist.tile([128, HEADS * RPC], F32, name="sbc_all")
            e02bc_all = persist.tile([128, HEADS * RPC], CDT, name="e02bc_all")

            # ============ one GAT layer ============
            def gat_layer(lidx, fin, r_dram, wa_dram, ht_loader):
                kch = _chunks(fin)
                nkc = len(kch)
                if ht_loader is None:
                    ht_sb = [ht_view(ci, sz) for ci, (lo, sz) in enumerate(kch)]
                else:
                    ht_sb = load_ht(ht_loader, kch)

                # --- s vector for all heads: [heads, own-i] ---
                ps_s = ps.tile([HEADS, RPC], F32, tag="p6")
                for ci, (lo, sz) in enumerate(kch):
                    wa_t = rstream.tile([128, HEADS], CDT, tag="wa", bufs=6)
                    nc.sync.dma_start(wa_t[:sz, :], wa_dram[lo:lo + sz, :])
                    _mm(nc, ps_s[:], wa_t[:sz, :], ht_sb[ci][:sz, :],
                        start=(ci == 0), stop=(ci == nkc - 1))
                nc.scalar.activation(s_sb[:], ps_s[:], AF.Copy)
                nc.scalar.activation(es02_sb[:], ps_s[:], AF.Exp, scale=ALPHA)
                for h in range(HEADS):
                    pb = ps.tile([128, RPC], F32, tag="p6", name=f"pb{h}")
                    nc.tensor.matmul(pb[:], eyef_sb[:, h * 128:(h + 1) * 128],
                                     s_sb[:], start=True, stop=True)
                    nc.scalar.activation(sbc_all[:, h * RPC:(h + 1) * RPC],
                                         pb[:], AF.Copy)
                    pb2 = ps.tile([128, RPC], F32, tag="p6", name=f"pb2{h}")
                    nc.tensor.matmul(pb2[:], eyec_sb[:, h * 128:(h + 1) * 128],
                                     es02_sb[:], start=True, stop=True)
                    nc.scalar.activation(e02bc_all[:, h * RPC:(h + 1) * RPC],
                                         pb2[:], AF.Copy)

                wg_list = []
                # --- per-head: local Wh -> DRAM -> AllGather ---
                for h in range(HEADS):
                    wl = dram.tile([RPC, HID + 2], CDT, name=f"wl{lidx}_{h}")
                    wg = dram.tile([N, HID + 2], CDT, name=f"wg{lidx}_{h}", addr_space="Shared")
                    wg_list.append(wg)
                    wl_sb = small.tile([128, NJT * (HID + 2)], CDT, tag="wl_sb")
                    psw_t = [ps.tile([128, HID + 1], F32, tag=f"p{jt}", name=f"psw{jt}")
                             for jt in range(NJT)]
                    for ci, (lo, sz) in enumerate(kch):
                        r_t = rstream.tile([128, HID + 1], CDT, tag="r", bufs=8)
                        nc.sync.dma_start(r_t[:sz, :], r_dram[h, lo:lo + sz, :])
                        for jt in range(NJT):
                            _mm(nc, psw_t[jt][:],
                                ht_sb[ci][:sz, jt * 128:(jt + 1) * 128],
                                r_t[:sz, :],
                                start=(ci == 0), stop=(ci == nkc - 1))
                    for jt in range(NJT):
                        base = jt * (HID + 2)
                        nc.vector.memset(wl_sb[:, base:base + 1], 1.0)
                        nc.scalar.activation(wl_sb[:, base + 1:base + HID + 2],
                                             psw_t[jt][:], AF.Copy)
                        nc.scalar.dma_start(wl[jt * 128:(jt + 1) * 128, :],
                                            wl_sb[:, base:base + HID + 2])
                    nc.gpsimd.collective_compute(
                        "AllGather", mybir.AluOpType.bypass, replica_groups=rg,
                        ins=[wl.opt()], outs=[wg.opt()])

                # --- per-head attention over gathered Wh ---
                # Software-pipelined: head h's divide/ELU/store is emitted
                # after head h+1's matmul phase so PE never starves at head
                # boundaries.
                W2 = HID + 2
                osp = [(0, 128), (128, 128), (256, 45)]

                def att_front(h):
                    wg = wg_list[h]
                    whb = whbufp.tile([128, NCH * W2], CDT, tag="whb",
                                      name=f"whb{h}")
                    nc.sync.dma_start(
                        whb[:].rearrange("p (c w) -> p c w", w=W2),
                        wg[:].rearrange("(c p) w -> p c w", p=128))
                    ed02 = small.tile([128, NCH], F32, tag="ed02",
                                      name=f"ed02_{h}")
                    dcols = whb[:].rearrange("p (c w) -> p c w", w=W2)[:, :, HID + 1]
                    nc.scalar.activation(ed02[:], dcols, AF.Exp, scale=ALPHA)
                    sbc = sbc_all[:, h * RPC:(h + 1) * RPC]
                    e02bc = e02bc_all[:, h * RPC:(h + 1) * RPC]

                    ps_att = [ps.tile([128, RPC], F32, tag=f"p{3 + om}",
                                      name=f"psatt{om}")
                              for om in range(len(osp))]
                    GRP = 3
                    for c0 in range(0, NCH, GRP):
                        a_t = ew.tile([128, GRP * RPC], CDT, tag="a", bufs=2)
                        b_t = ew.tile([128, GRP * RPC], CDT, tag="b", bufs=2)
                        for k in range(GRP):
                            c = c0 + k
                            nc.scalar.activation(
                                a_t[:, k * RPC:(k + 1) * RPC], sbc, AF.Exp,
                                bias=whb[:, c * W2 + HID + 1:c * W2 + HID + 2])
                            nc.vector.tensor_scalar_mul(
                                b_t[:, k * RPC:(k + 1) * RPC], e02bc,
                                ed02[:, c:c + 1])
                        nc.vector.tensor_max(b_t[:], a_t[:], b_t[:])
                        p_t = ew.tile([128, GRP * RPC], CDT, tag="p", bufs=5)
                        nc.vector.tensor_mul(
                            p_t[:], b_t[:],
                            adj_sb[:, c0 * RPC:(c0 + GRP) * RPC])
                        for k in range(GRP):
                            c = c0 + k
                            for om, (lo, sz) in enumerate(osp):
                                _mm(nc, ps_att[om][:sz, :],
                                    whb[:, c * W2 + lo:c * W2 + lo + sz],
                                    p_t[:, k * RPC:(k + 1) * RPC],
                                    start=(c == 0), stop=(c == NCH - 1))
                    # free PSUM fast: copy raw sums to SBUF
                    att32 = [ew.tile([128, RPC], F32, tag=f"att32_{om}",
                                     name=f"att32_{om}", bufs=2)
                             for om in range(len(osp))]
                    with tc.high_priority():
                        for om, (lo, sz) in enumerate(osp):
                            nc.scalar.activation(att32[om][:sz, :],
                                                 ps_att[om][:sz, :], AF.Copy)
                    return att32

                def att_back(h, att32):
                    scr = small.tile([1, RPC], F32, tag="scr")
                    rden = small.tile([1, RPC], F32, tag="rden")
                    nc.vector.reciprocal_approx_accurate(
                        rden[:], att32[0][0:1, :], scr[:])
                    rbc = ps.tile([128, RPC], F32, tag="p7", name="rbc")
                    nc.tensor.matmul(rbc[:], onesf_sb[:], rden[:],
                                     start=True, stop=True)
                    # om0 rows: 0=denominator junk, 1..127 = Wh outs 0..126
                    store = [(0, 0, 128, 1), (1, 127, 128, 0), (2, 255, 45, 0)]
                    for om, hlo, rows, roff in store:
                        y_t = ew.tile([128, RPC], CDT, tag="y")
                        nc.vector.tensor_mul(y_t[:rows, :], att32[om][:rows, :],
                                             rbc[:rows, :])
                        q_t = ew.tile([128, RPC], CDT, tag="q")
                        nc.scalar.activation(q_t[:rows, :], y_t[:rows, :], AF.Exp)
                        nc.vector.tensor_scalar(q_t[:rows, :], q_t[:rows, :],
                                                -1.0, 0.0, mybir.AluOpType.add,
                                                mybir.AluOpType.min)
                        nc.vector.tensor_max(y_t[:rows, :], y_t[:rows, :],
                                             q_t[:rows, :])
                        store_hT(y_t, roff, rows, h * HID + hlo)

                pend = None
                for h in range(HEADS):
                    a32 = att_front(h)
                    if pend is not None:
                        att_back(*pend)
                    pend = (h, a32)
                att_back(*pend)
                return wg_list[0]

            def _tail():
                # ---------------- output attention layer ----------------
                FO = HEADS * HID
                kch = _chunks(FO)
                nkc = len(kch)
                ht_sb = [ht_view(ci, sz) for ci, (lo, sz) in enumerate(kch)]
                # s vector
                ps_so = ps.tile([1, RPC], F32, tag="p6")
                for ci, (lo, sz) in enumerate(kch):
                    wa_t = rstream.tile([128, 1], CDT, tag="wa", bufs=6)
                    nc.sync.dma_start(wa_t[:sz, :], WAo[lo:lo + sz, :])
                    _mm(nc, ps_so[:], wa_t[:sz, :], ht_sb[ci][:sz, :],
                        start=(ci == 0), stop=(ci == nkc - 1))
                so_sb = persist.tile([1, RPC], F32, name="so_sb")
                nc.scalar.activation(so_sb[:], ps_so[:], AF.Copy)
                eso02 = persist.tile([1, RPC], CDT, name="eso02")
                nc.scalar.activation(eso02[:], ps_so[:], AF.Exp, scale=ALPHA)

                # local Wh_out [RPC, 602] -> 3 AllGathers (one per own j-tile)
                WO2 = OUT_ATT + 2
                wlo_p = [dram.tile([128, WO2], CDT, name=f"wlo{jt}")
                         for jt in range(NJT)]
                wgo_p = [dram.tile([N // NJT, WO2], CDT, name=f"wgo{jt}",
                                   addr_space="Shared") for jt in range(NJT)]
                nsp = _chunks(OUT_ATT + 1, 512)  # [(0,512),(512,89)]
                wlo_sb = small.tile([128, NJT * WO2], CDT, tag="wlo_sb")
                for jt in range(NJT):
                    pswo = [ps.tile([128, sz], F32, tag=f"p{si}", name=f"pswo{si}")
                            for si, (lo, sz) in enumerate(nsp)]
                    for ci, (lo, sz) in enumerate(kch):
                        r_t = rstream.tile([128, OUT_ATT + 1], CDT, tag="ro")
                        nc.sync.dma_start(r_t[:sz, :], Ro[lo:lo + sz, :])
                        for si, (slo, ssz) in enumerate(nsp):
                            _mm(nc, pswo[si][:],
                                ht_sb[ci][:sz, jt * 128:(jt + 1) * 128],
                                r_t[:sz, slo:slo + ssz],
                                start=(ci == 0), stop=(ci == nkc - 1))
                    base = jt * WO2
                    for si, (slo, ssz) in enumerate(nsp):
                        nc.scalar.activation(wlo_sb[:, base + slo:base + slo + ssz],
                                             pswo[si][:], AF.Copy)
                    nc.vector.memset(wlo_sb[:, base + WO2 - 1:base + WO2], 1.0)
                    nc.scalar.dma_start(wlo_p[jt][:], wlo_sb[:, base:base + WO2])
                    nc.gpsimd.collective_compute(
                        "AllGather", mybir.AluOpType.bypass, replica_groups=rg,
                        ins=[wlo_p[jt].opt()], outs=[wgo_p[jt].opt()])

                # attention (single head), row-major output for pooling
                pbo = ps.tile([128, RPC], F32, tag="p6", name="pbo")
                nc.tensor.matmul(pbo[:], onesf_sb[:], so_sb[:],
                                 start=True, stop=True)
                sbco = ew.tile([128, RPC], F32, tag="sbc")
                nc.scalar.activation(sbco[:], pbo[:], AF.Copy)
                pbo2 = ps.tile([128, RPC], F32, tag="p6", name="pbo2")
                nc.tensor.matmul(pbo2[:], onesc_sb[:], eso02[:],
                                 start=True, stop=True)
                e02bco = ew.tile([128, RPC], CDT, tag="e02bc")
                nc.scalar.activation(e02bco[:], pbo2[:], AF.Copy)

                onsp = _chunks(OUT_ATT + 2, 512)  # [(0,512),(512,90)]
                ps_o = [[ps.tile([128, sz], F32, tag=f"p{it * 2 + si}", name=f"pso{it}_{si}")
                         for si, (lo, sz) in enumerate(onsp)] for it in range(NJT)]
                GRP = 3
                for c0 in range(0, NCH, GRP):
                    whcs = []
                    a_t = ew.tile([128, GRP * RPC], CDT, tag="a", bufs=2)
                    b_t = ew.tile([128, GRP * RPC], CDT, tag="b", bufs=2)
                    for k in range(GRP):
                        c = c0 + k
                        whc = rstream.tile([128, WO2], CDT, tag=f"whc{k}",
                                           name=f"whc{c}", bufs=2)
                        nc.sync.dma_start(
                            whc[:],
                            wgo_p[c % NJT][(c // NJT) * 128:(c // NJT + 1) * 128, :])
                        whcs.append(whc)
                        edo02c = small.tile([128, 1], F32, tag="ed02c", bufs=3)
                        nc.scalar.activation(edo02c[:], whc[:, OUT_ATT:OUT_ATT + 1],
                                             AF.Exp, scale=ALPHA)
                        nc.scalar.activation(
                            a_t[:, k * RPC:(k + 1) * RPC], sbco[:], AF.Exp,
                            bias=whc[:, OUT_ATT:OUT_ATT + 1])
                        nc.vector.tensor_scalar_mul(
                            b_t[:, k * RPC:(k + 1) * RPC], e02bco[:], edo02c[:])
                    nc.vector.tensor_max(b_t[:], a_t[:], b_t[:])
                    p_t = ew.tile([128, GRP * RPC], CDT, tag="p", bufs=5)
                    nc.vector.tensor_mul(p_t[:], b_t[:],
                                         adj_sb[:, c0 * RPC:(c0 + GRP) * RPC])
                    for k in range(GRP):
                        c = c0 + k
                        for it in range(NJT):
                            for si, (slo, ssz) in enumerate(onsp):
                                _mm(nc, ps_o[it][si][:],
                                    p_t[:, k * RPC + it * 128:k * RPC + (it + 1) * 128],
                                    whcs[k][:, slo:slo + ssz],
                                    start=(c == 0), stop=(c == NCH - 1))

                # divide + ELU + pooled^T partial via matmul with smat
                gsp = _chunks(OUT_ATT)  # 600 -> 128x4 + 88
                att_tiles = []
                for it in range(NJT):
                    denc = small.tile([128, 1], F32, tag="denc")
                    scrc = small.tile([128, 1], F32, tag="scrc")
                    rdenc = small.tile([128, 1], F32, tag="rdenc")
                    nc.vector.tensor_copy(denc[:], ps_o[it][-1][:, 89:90])
                    nc.vector.reciprocal_approx_accurate(rdenc[:], denc[:], scrc[:])
                    att_sb = ew.tile([128, OUT_ATT], CDT, tag=f"atts{it}",
                                     name=f"atts{it}", bufs=1)
                    for si, (slo, ssz) in enumerate(onsp):
                        cols = min(ssz, OUT_ATT - slo)
                        y = att_sb[:, slo:slo + cols]
                        nc.vector.tensor_scalar_mul(y, ps_o[it][si][:, :cols], rdenc[:])
                        q_t = ew.tile([128, OUT_ATT], CDT, tag="qo")
                        nc.scalar.activation(q_t[:, :cols], y, AF.Exp)
                        nc.vector.tensor_scalar(q_t[:, :cols], q_t[:, :cols],
                                                -1.0, 0.0, mybir.AluOpType.add,
                                                mybir.AluOpType.min)
                        nc.vector.tensor_max(y, y, q_t[:, :cols])
                    if DEBUG_STAGE == "HO":
                        a32 = ew.tile([128, OUT_ATT], F32, tag="a32dbg")
                        nc.vector.tensor_copy(a32[:], att_sb[:])
                        nc.sync.dma_start(dbg[it * 128:(it + 1) * 128, :], a32[:])
                    att_tiles.append(att_sb)

                # pooled^T [600, 96] -> AllReduce
                pool_l = dram.tile([OUT_ATT, N_GRAPHS], F32, name="pool_l")
                pool_g = dram.tile([OUT_ATT, N_GRAPHS], F32, name="pool_g", addr_space="Shared")
                for g, (glo, gsz) in enumerate(gsp):
                    psp = ps.tile([128, N_GRAPHS], F32, tag="p7", name=f"psp{g}")
                    for it in range(NJT):
                        _mm(nc, psp[:gsz, :], att_tiles[it][:, glo:glo + gsz],
                            smat_sb[it][:], start=(it == 0), stop=(it == NJT - 1))
                    pl_sb = small.tile([128, N_GRAPHS], F32, tag="pl_sb")
                    nc.scalar.activation(pl_sb[:gsz, :], psp[:gsz, :], AF.Copy)
                    nc.sync.dma_start(pool_l[glo:glo + gsz, :], pl_sb[:gsz, :])
                nc.gpsimd.collective_compute(
                    "AllReduce", mybir.AluOpType.add, replica_groups=rg,
                    ins=[pool_l.opt()], outs=[pool_g.opt()])
                if DEBUG_STAGE == "POOL":
                    nc.sync.dma_start(dbg[:], pool_g[:])

                # ---------------- MLP (replicated on every core) ----------------
                pg_sb = []
                for g, (glo, gsz) in enumerate(gsp):
                    t32 = small.tile([128, N_GRAPHS], F32, tag="pg32")
                    nc.sync.dma_start(t32[:gsz, :], pool_g[glo:glo + gsz, :])
                    t = persist.tile([128, N_GRAPHS], CDT, name=f"pg{g}")
                    nc.vector.tensor_copy(t[:gsz, :], t32[:gsz, :])
                    pg_sb.append(t)
                wm1_sb = []
                for g, (glo, gsz) in enumerate(gsp):
                    w = persist.tile([128, MLP_HID], CDT, name=f"wm1_{g}")
                    nc.sync.dma_start(w[:gsz, :], Wm1[glo:glo + gsz, :])
                    wm1_sb.append(w)
                msp = _chunks(MLP_HID)
                bm1_sb = []
                for m, (mlo, msz) in enumerate(msp):
                    b = persist.tile([128, 1], F32, name=f"bm1_{m}")
                    nc.sync.dma_start(b[:msz, :], bm1[mlo:mlo + msz, :])
                    bm1_sb.append(b)
                h1_sb = []
                for m, (mlo, msz) in enumerate(msp):
                    psm = ps.tile([128, N_GRAPHS], F32, tag="p7")
                    for g, (glo, gsz) in enumerate(gsp):
                        _mm(nc, psm[:msz, :], wm1_sb[g][:gsz, mlo:mlo + msz],
                            pg_sb[g][:gsz, :], start=(g == 0), stop=(g == len(gsp) - 1))
                    t = persist.tile([128, N_GRAPHS], CDT, name=f"h1_{m}")
                    nc.scalar.activation(t[:msz, :], psm[:msz, :], AF.Relu,
                                         bias=bm1_sb[m][:msz, :])
                    h1_sb.append(t)
                wm2_sb = []
                for m, (mlo, msz) in enumerate(msp):
                    w = persist.tile([128, NOUT], CDT, name=f"wm2_{m}")
                    nc.sync.dma_start(w[:msz, :], Wm2[mlo:mlo + msz, :])
                    wm2_sb.append(w)
                for o, (olo, osz) in enumerate(_chunks(NOUT)):
                    b2 = small.tile([128, 1], F32, tag="bm2")
                    nc.sync.dma_start(b2[:osz, :], bm2[olo:olo + osz, :])
                    psm = ps.tile([128, N_GRAPHS], F32, tag="p7")
                    for m, (mlo, msz) in enumerate(msp):
                        _mm(nc, psm[:osz, :], wm2_sb[m][:msz, olo:olo + osz],
                            h1_sb[m][:msz, :], start=(m == 0), stop=(m == len(msp) - 1))
                    ot = small.tile([128, N_GRAPHS], F32, tag="ot")
                    nc.vector.tensor_scalar_add(ot[:osz, :], psm[:osz, :], b2[:osz, :])
                    nc.sync.dma_start(outT[olo:olo + osz, :], ot[:osz, :])


            # ---------------- layers 0, 1 ----------------
            def dump_ht():
                for ci, (lo, sz) in enumerate(_chunks(HEADS * HID)):
                    t32 = rstream.tile([128, RPC], F32, tag="tdb32", name=f"t32{ci}")
                    nc.vector.tensor_copy(t32[:sz, :], ht_view(ci, sz))
                    nc.sync.dma_start(dbg[lo:lo + sz, :], t32[:sz, :])

            wg00 = gat_layer(0, F_IN, R0, WA0,
                      lambda t, lo, sz: nc.sync.dma_start(t[:sz, :], xT[lo:lo + sz, :]))
            stop = False
            if DEBUG_STAGE == "L0":
                dump_ht()
                stop = True
            elif DEBUG_STAGE == "S0":
                s32 = persist.tile([HEADS, RPC], F32, name="s32dbg")
                nc.vector.tensor_copy(s32[:], s_sb[:])
                nc.sync.dma_start(dbg[:], s32[:])
                stop = True
            elif DEBUG_STAGE == "WG0":
                for ci in range(NCH):
                    tdb = rstream.tile([128, HID + 1], CDT, tag="tdb", name=f"tdb{ci}")
                    nc.sync.dma_start(tdb[:], wg00[ci * 128:(ci + 1) * 128, :])
                    t32 = rstream.tile([128, HID + 1], F32, tag="tdb32", name=f"t32{ci}")
                    nc.vector.tensor_copy(t32[:], tdb[:])
                    nc.sync.dma_start(dbg[ci * 128:(ci + 1) * 128, :], t32[:])
                stop = True
            if not stop:
                gat_layer(1, HEADS * HID, R1, WA1, None)
                if DEBUG_STAGE == "L1":
                    dump_ht()
                    stop = True
            if not stop:
                _tail()

    nc.compile()
    return nc


# ======================= host side =======================

def _np_cdt(a):
    if CDT == F32:
        return np.ascontiguousarray(a, dtype=np.float32)
    import ml_dtypes
    return np.ascontiguousarray(np.asarray(a, np.float32).astype(ml_dtypes.bfloat16))


def _prep_inputs(x, edge_index, batch, W0, a0_src, a0_dst, W1, a1_src, a1_dst,
                 W_out, ao_src, ao_dst, Wm1, bm1, Wm2, bm2):
    x = np.asarray(x, np.float32)
    ei = np.asarray(edge_index)
    batch = np.asarray(batch).astype(np.int64)
    adj = np.zeros((N, N), np.float32)
    adj[ei[0], ei[1]] = 1.0

    cnt = np.bincount(batch, minlength=N_GRAPHS).astype(np.float32)
    cnt = np.maximum(cnt, 1.0)
    smat_full = np.zeros((N, N_GRAPHS), np.float32)
    smat_full[np.arange(N), batch] = 1.0 / cnt[batch]

    W0 = np.asarray(W0, np.float32)
    W1 = np.asarray(W1, np.float32)
    W_out = np.asarray(W_out, np.float32)

    def fuse(W, a_dst):  # [H, F, O], [H, O] -> [H, F, O+1]
        wad = np.einsum('hfo,ho->hf', W, np.asarray(a_dst, np.float32))
        return np.concatenate([W, wad[:, :, None]], axis=2)

    R0p = fuse(W0, a0_dst)
    WA0p = np.einsum('hfo,ho->fh', W0, np.asarray(a0_src, np.float32))
    R1p = fuse(W1, a1_dst)
    WA1p = np.einsum('hfo,ho->fh', W1, np.asarray(a1_src, np.float32))
    Rop = np.concatenate(
        [W_out, (W_out @ np.asarray(ao_dst, np.float32))[:, None]], axis=1)
    WAop = (W_out @ np.asarray(ao_src, np.float32))[:, None]

    shared = dict(
        R0=_np_cdt(R0p), WA0=_np_cdt(WA0p), R1=_np_cdt(R1p), WA1=_np_cdt(WA1p),
        Ro=_np_cdt(Rop), WAo=_np_cdt(WAop),
        Wm1=_np_cdt(Wm1), bm1=np.ascontiguousarray(np.asarray(bm1, np.float32)[:, None]),
        Wm2=_np_cdt(Wm2), bm2=np.ascontiguousarray(np.asarray(bm2, np.float32)[:, None]),
    )
    eye = np.kron(np.eye(HEADS, dtype=np.float32), np.ones((1, 128), np.float32))
    shared["eye10f"] = np.ascontiguousarray(eye)
    shared["eye10c"] = _np_cdt(eye)
    xT_full = x.T
    in_maps = []
    for c in range(NCORES):
        rows = slice(c * RPC, (c + 1) * RPC)
        m = dict(shared)
        m["xT"] = _np_cdt(xT_full[:, rows])
        m["adjT"] = _np_cdt(adj[rows, :].T)
        m["smat"] = _np_cdt(smat_full[rows, :])
        in_maps.append(m)
    return in_maps


_last_results = None


def kernel(**inputs):
    global _last_results
    if "k" not in _compiled:
        _compiled["k"] = build()
    nc = _compiled["k"]
    in_maps = _prep_inputs(**inputs)
    kw = {}
    if TRACE:
        import tracehook
        tracehook.install()
        kw = dict(trace=True)
        td = os.environ.get("KERNEL_TRACEDIR")
        if td:
            kw["tmpdir"] = td
    res = run_bass_kernel_spmd(nc, in_maps, core_ids=list(range(NCORES)), **kw)
    _last_results = res
    return np.ascontiguousarray(res.results[0]["outT"].T)

